# revision 36
# baseline (speedup 1.0000x reference)
"""Trainium2 Bass kernel for the attention layer:

    f = wf@x+bf; g = wg@x+bg; h = wh@x+bh            (1x1 convs, Ci=32)
    attn = softmax(f^T g, axis=-1)                   (per batch, N=4096)
    out = (wv @ (h @ attn^T) + bv) * gamma + x

Sharding: 8 cores = 4 batches x 2 query-halves (2048 queries each).
Each core receives the full (256, 4096) batch slice with its query half
permuted to the front, so the SPMD program uses fixed offsets.

Final design (engine-balance oriented; every PSUM->SBUF byte must cross
ACT or DVE at 1 elem/cycle/lane, so those two engines are co-critical):
  - exp of the 2048x4096 logits is SPLIT between the Scalar engine
    (exact table exp) and the Vector engine (Schraudolph bit-trick in
    bf16: eT_bits = int16(logits * 2^7/ln2 + B), one fused
    tensor_scalar; ~3.3% per-element error washes out to ~3e-3
    end-to-end through the softmax).
  - x0 = [h; den]@attn^T is 2x column-tiled: strip A (psum partitions
    0-32) handles queries 0-255 of the chunk, strip B (partitions
    64-96) queries 256-511 -- concurrent in the PE.  The ones column
    of hT sits LAST, so the denominators land on partitions 32/96.
  - denominators are staged to partition 0 via two tiny SBUF DMAs;
    one DVE bit-trick reciprocal seed + a DVE Newton step (TT + STT)
    yields -1/den for both strips in one row; gpsimd
    partition_broadcast (partition 0 -> 0..31, the only working form)
    replicates it.  The v weights are negated on the host to absorb
    the Newton sign.
  - strip B's channels are DMA-moved to partitions 0-31 so both
    normalize muls / v matmuls run at partition base 0 (base-64
    contractions and non-0 broadcasts are broken in this stack).
  - v bias (gamma*(bv+wv@bh)) + residual are fused into one DVE
    scalar_tensor_tensor during the mandatory vps PSUM->SBUF transit.
  - v-projection matmuls of chunk q are emitted in the middle of chunk
    q+1's logits stream so the PE never idles on the normalize chain
    (keeps the HAM clock gate at 8/8).
  - hT blocks are staged 16-per-PSUM-bank so their SBUF transit is two
    512-wide copies instead of 32 tiny ones.
  - x is loaded once, in bf16; all matmuls except logits run bf16.
"""

import os
import numpy as np
import ml_dtypes

import concourse.bass as bass
import concourse.mybir as mybir
import concourse.tile as tile
from concourse import bacc
from concourse.bass import ts
from concourse.bass_utils import run_bass_kernel_spmd

F32 = mybir.dt.float32
F32R = mybir.dt.float32r
BF16 = mybir.dt.bfloat16
I32 = mybir.dt.int32
I16 = mybir.dt.int16
EXP = mybir.ActivationFunctionType.Exp
COPY = mybir.ActivationFunctionType.Copy
IDENT = mybir.ActivationFunctionType.Identity
MULT = mybir.AluOpType.mult
ADD = mybir.AluOpType.add
SUB = mybir.AluOpType.subtract

B, C, W, H = 4, 256, 64, 64
N = W * H            # 4096 keys/queries per batch
CI = 32              # inner channels
NCORES = 8
NQ = N // 2          # queries per core
QC = 512             # query chunk = one fp32 PSUM bank
QH = QC // 2         # per-strip query half
NQC = NQ // QC       # 4 query chunks per core
KC = 128             # key chunk = partition dim
NKC = N // KC        # 32 key chunks
GRP = 2              # key chunks per exp group (PSUM banks per tile)
NG = NKC // GRP      # 16 groups per query chunk
NWARM = 55           # dummy matmuls bridging the input DMA (~18us)

# Schraudolph exp constants for bf16 output:
#   exp(x) ~= bitcast_bf16(int16(x * 2^7/ln2 + (127*2^7 - 5.5)))
SCH_A = 184.66497
SCH_B = 16250.5
# Bit-trick reciprocal seed: 1/x ~= bitcast_f32(MAGIC - bits(x))
RCP_MAGIC = float(0x7EF127EA)

# Which exp groups (by index within a chunk) run on ACT (exact exp);
# the rest run on DVE (Schraudolph).  Tuned for engine balance.
EXP_ACT = {0, 2, 4, 6, 8, 10, 12, 14, 15}

# Trace knob for test harnesses: set kernel.TRACE = True to profile.
TRACE = False
LAST_EXEC_NS = None

_cached_nc = None


def _build():
    nc = bacc.Bacc(
        "TRN2", target_bir_lowering=False, debug=False, num_devices=NCORES
    )
    xbf_d = nc.dram_tensor("xbf", (C, N), BF16, kind="ExternalInput").ap()
    wfT_d = nc.dram_tensor("wfT", (C, 128), BF16, kind="ExternalInput").ap()
    wgT_d = nc.dram_tensor("wgT", (C, 128), BF16, kind="ExternalInput").ap()
    whT_d = nc.dram_tensor("whT", (C, CI), BF16, kind="ExternalInput").ap()
    wv2_d = nc.dram_tensor("wv2", (CI, C), BF16, kind="ExternalInput").ap()
    bv2_d = nc.dram_tensor("bv2", (128, 2), F32, kind="ExternalInput").ap()
    bf_d = nc.dram_tensor("bf", (128, 1), F32, kind="ExternalInput").ap()
    bg_d = nc.dram_tensor("bg", (128, 1), F32, kind="ExternalInput").ap()
    out_d = nc.dram_tensor("out", (C, NQ), F32, kind="ExternalOutput").ap()
    rscr_d = nc.dram_tensor("rscr", (NQC, QC), F32, kind="Internal").ap()

    xbfr = xbf_d.rearrange("(cc p) n -> p cc n", p=128)
    outr = out_d.rearrange("(oc p) n -> p oc n", p=128)

    with tile.TileContext(nc) as tc:
        with (
            tc.tile_pool(name="consts", bufs=1) as consts,
            tc.tile_pool(name="data", bufs=1) as data,
            tc.tile_pool(name="eTp", bufs=6) as eTp,
            tc.tile_pool(name="nrm", bufs=2) as nrm,
            tc.tile_pool(name="outp", bufs=3) as outp,
            tc.tile_pool(name="pl", bufs=2, space="PSUM") as pl,
            tc.tile_pool(name="pp", bufs=2, space="PSUM") as pp,
            tc.tile_pool(name="px0", bufs=2, space="PSUM") as px0,
        ):
            # ---- engine warm-ups (overlap the input DMAs) ----
            scratch = consts.tile([128, QC], F32)
            nc.vector.memset(scratch, 0.0)
            scratchR = consts.tile([128, QC], F32R)
            nc.vector.tensor_copy(scratchR, scratch)
            wps = pp.tile([128, QC], F32, tag="pp")
            for i in range(NWARM):
                nc.tensor.matmul(
                    wps, lhsT=scratchR[:, 0:128], rhs=scratchR,
                    start=True, stop=True, skip_group_check=True,
                )
            wsc = consts.tile([4, 16], F32)
            nc.scalar.activation(out=wsc[0:1, 0:8], in_=scratch[0:1, 0:8],
                                 func=EXP)
            nc.vector.tensor_scalar(
                out=wsc[0:1, 8:16].bitcast(I16)[:, 0:8],
                in0=scratch[0:1, 8:16],
                scalar1=SCH_A, scalar2=SCH_B, op0=MULT, op1=ADD,
            )
            wsc2 = consts.tile([4, 24], F32)
            wsc3 = consts.tile([4, 24], F32)
            nc.vector.memset(wsc2[0:1, :], 1.0)
            nc.gpsimd.tensor_mul(
                wsc3[0:1, 0:8], wsc2[0:1, 0:8], wsc2[0:1, 8:16]
            )
            nc.gpsimd.tensor_sub(
                wsc3[0:1, 8:16], wsc3[0:1, 0:8], wsc2[0:1, 16:24]
            )

            # ---- constants ----
            wfT_sb = consts.tile([128, 2, 128], BF16)
            nc.sync.dma_start(
                out=wfT_sb, in_=wfT_d.rearrange("(cc p) o -> p cc o", p=128)
            )
            wgT_sb = consts.tile([128, 2, 128], BF16)
            nc.sync.dma_start(
                out=wgT_sb, in_=wgT_d.rearrange("(cc p) o -> p cc o", p=128)
            )
            whT_sb = consts.tile([128, 2, CI], BF16)
            nc.sync.dma_start(
                out=whT_sb, in_=whT_d.rearrange("(cc p) o -> p cc o", p=128)
            )
            wv2_sb = consts.tile([CI, 2, 128], BF16)
            nc.sync.dma_start(
                out=wv2_sb, in_=wv2_d.rearrange("p (oc m) -> p oc m", oc=2)
            )
            bv2_sb = consts.tile([128, 2], F32)
            nc.sync.dma_start(out=bv2_sb, in_=bv2_d)
            bf_sb = consts.tile([128, 1], F32)
            nc.sync.dma_start(out=bf_sb, in_=bf_d)
            bg_sb = consts.tile([128, 1], F32)
            nc.sync.dma_start(out=bg_sb, in_=bg_d)
            ones_sb = consts.tile([128, 1], F32)
            nc.vector.memset(ones_sb, 1.0)
            twos32_sb = consts.tile([33, QC], F32)
            nc.vector.memset(twos32_sb[32:33, :], 2.0)

            # ---- x (bf16 only) ----
            xbf_sb = data.tile([128, 2, N], BF16)
            for s in range(4):
                nc.sync.dma_start(
                    out=xbf_sb[:, :, ts(s, N // 4)],
                    in_=xbfr[:, :, ts(s, N // 4)],
                )

            # ---- f, g (replicated on 4 strips), hT ----
            f_sb = data.tile([128, NQ], BF16)
            g_sb = data.tile([128, N], BF16)
            # hT layout: channels first, ones column LAST -> the x0
            # denominator lands on partitions 32 / 96.
            hT_sb = data.tile([128, NKC, CI + 1], BF16)
            nc.vector.tensor_copy(
                hT_sb[:, :, CI : CI + 1], ones_sb.to_broadcast([128, NKC, 1])
            )

            def emit_f(j):
                ps = pp.tile([128, QC], F32, tag="pp", name=f"psf{j}")
                for cc in range(2):
                    nc.tensor.matmul(
                        ps, lhsT=wfT_sb[:, cc, :],
                        rhs=xbf_sb[:, cc, ts(j, QC)],
                        start=cc == 0, stop=cc == 1,
                    )
                nc.scalar.activation(
                    out=f_sb[:, ts(j, QC)], in_=ps, func=IDENT, bias=bf_sb
                )

            def emit_g(j):
                ps = pp.tile([128, QC], F32, tag="pp", name=f"psg{j}")
                for cc in range(2):
                    nc.tensor.matmul(
                        ps, lhsT=wgT_sb[:, cc, :],
                        rhs=xbf_sb[:, cc, ts(j, QC)],
                        start=cc == 0, stop=cc == 1,
                    )
                nc.scalar.activation(
                    out=g_sb[:, ts(j, QC)], in_=ps, func=IDENT, bias=bg_sb
                )

            def emit_hT_round(r):
                # 16 hT k-blocks staged into one 512-wide PSUM bank,
                # then one wide transit into hT_sb.
                ps = pp.tile([128, QC], F32, tag="pp", name=f"psh{r}")
                for kl in range(16):
                    kc = 16 * r + kl
                    for cc in range(2):
                        nc.tensor.matmul(
                            ps[:, ts(kl, CI)],
                            lhsT=xbf_sb[:, cc, ts(kc, KC)],
                            rhs=whT_sb[:, cc, :],
                            start=cc == 0, stop=cc == 1,
                        )
                nc.scalar.activation(
                    out=hT_sb[:, ts(r, 16), 0:CI],
                    in_=ps.rearrange("p (k c) -> p k c", c=CI), func=COPY,
                )

            # pending work carried across the chunk loop for pipelining
            pend = {}

            def emit_norm_copies(qi):
                # den rows + strip-B channels off PSUM, then partition
                # moves via DMA.
                x0t = pend["x0t"]
                den = nrm.tile([128, QH], F32, tag="den")
                nc.scalar.activation(out=den[32:33, :], in_=x0t[32:33, :],
                                     func=COPY)
                nc.scalar.activation(out=den[96:97, :], in_=x0t[96:97, :],
                                     func=COPY)
                x0cB = nrm.tile([128, QH], F32, tag="x0cB")
                nc.scalar.activation(out=x0cB[64:96, :], in_=x0t[64:96, :],
                                     func=COPY)
                dd = nrm.tile([1, 2 * QH], F32, tag="dd")
                nc.sync.dma_start(out=dd[0:1, 0:QH], in_=den[32:33, :])
                nc.sync.dma_start(out=dd[0:1, QH:], in_=den[96:97, :])
                x0b = nrm.tile([128, QH], F32, tag="x0b")
                nc.sync.dma_start(out=x0b[0:32, :], in_=x0cB[64:96, :])
                pend["dd"] = dd
                pend["x0b"] = x0b

            def emit_norm_recip(qi, fast=False):
                # DVE bit-trick seed + DVE Newton step -> -1/den for
                # both strips in one partition-0 row, then gp broadcasts
                # (partition 0 -> 0..31, the only working form).
                dd = pend.pop("dd")
                rr = nrm.tile([1, 2 * QH], F32, tag="rr")
                nc.vector.tensor_scalar(
                    out=rr.bitcast(I32), in0=dd.bitcast(I32),
                    scalar1=-1.0, scalar2=RCP_MAGIC, op0=MULT, op1=ADD,
                )
                r1 = nrm.tile([1, 2 * QH], F32, tag="r1")
                if fast:
                    # seed-only -1/den (max ~6% den error on this chunk
                    # alone; ~1e-3 end-to-end) -- negate to match the
                    # Newton sign convention.
                    nc.vector.tensor_scalar(
                        out=r1, in0=rr, scalar1=-1.0, scalar2=0.0,
                        op0=MULT, op1=ADD,
                    )
                else:
                    nc.vector.tensor_mul(r1, dd, rr)
                    nc.vector.scalar_tensor_tensor(
                        r1, r1, 2.0, rr, op0=SUB, op1=MULT,
                    )
                rcpbA = nrm.tile([32, QH], F32, tag="rcpbA")
                nc.gpsimd.partition_broadcast(rcpbA, r1[0:1, 0:QH])
                rcpbB = nrm.tile([32, QH], F32, tag="rcpbB")
                nc.gpsimd.partition_broadcast(rcpbB, r1[0:1, QH:])
                pend["rcpbA"] = rcpbA
                pend["rcpbB"] = rcpbB

            def emit_norm_mul(qi):
                x0t, x0b = pend["x0t"], pend.pop("x0b")
                rcpbA, rcpbB = pend.pop("rcpbA"), pend.pop("rcpbB")
                x0a = nrm.tile([32, QH], BF16, tag="x0a")
                nc.vector.tensor_mul(x0a, x0t[0:32, :], rcpbA)
                x0ab = nrm.tile([32, QH], BF16, tag="x0ab")
                nc.vector.tensor_mul(x0ab, x0b[0:32, :], rcpbB)
                pend["x0a"] = x0a
                pend["x0ab"] = x0ab

            def emit_v_out(qi):
                x0a, x0ab = pend.pop("x0a"), pend.pop("x0ab")
                pend.pop("x0t")
                for oc in range(2):
                    vps = pp.tile([128, QC], F32, tag="pp", name=f"v{qi}{oc}")
                    nc.tensor.matmul(
                        vps[:, 0:QH], lhsT=wv2_sb[:, oc, :], rhs=x0a,
                        start=True, stop=True, skip_group_check=True,
                    )
                    nc.tensor.matmul(
                        vps[:, QH:QC], lhsT=wv2_sb[:, oc, :], rhs=x0ab,
                        start=True, stop=True, skip_group_check=True,
                    )
                    ot = outp.tile([128, QC], F32)
                    nc.vector.scalar_tensor_tensor(
                        ot, vps, bv2_sb[:, oc : oc + 1],
                        xbf_sb[:, oc, ts(qi, QC)], op0=ADD, op1=ADD,
                    )
                    nc.sync.dma_start(out=outr[:, oc, ts(qi, QC)], in_=ot)

            emit_f(0)
            emit_g(0)

            # ---- main loop over query chunks ----
            for qi in range(NQC):
                x0q = []
                x0t = px0.tile([128, QH], F32, tag="x0")

                def emit_x0(g0, eT):
                    for j in range(GRP):
                        kc = g0 + j
                        st, sp = kc == 0, kc == NKC - 1
                        nc.tensor.matmul(
                            x0t[0:33, :], lhsT=hT_sb[:, kc, :],
                            rhs=eT[:, j, 0:QH], start=st, stop=sp,
                            tile_position=(0, 0), skip_group_check=True,
                        )
                        nc.tensor.matmul(
                            x0t[64:97, :], lhsT=hT_sb[:, kc, :],
                            rhs=eT[:, j, QH:QC], start=st, stop=sp,
                            tile_position=(0, 64), skip_group_check=True,
                        )

                for gi in range(NG):
                    if qi == 0:
                        # JIT emission of f/g/hT so the PE stream
                        # interleaves them with chunk 0's work.
                        if gi == 0:
                            emit_hT_round(0)
                            emit_g(1)
                        if gi % 2 == 0 and 2 <= gi // 2 + 2 <= 7:
                            emit_g(gi // 2 + 2)
                        if gi == 4:
                            emit_hT_round(1)
                        if gi in (11, 13, 15):
                            emit_f((gi - 9) // 2)
                    else:
                        # pipelined tail of the previous chunk
                        if gi == 2:
                            emit_norm_recip(qi - 1)
                        if gi == 6:
                            emit_norm_mul(qi - 1)
                        if gi == 10:
                            emit_v_out(qi - 1)
                        if gi == 0:
                            pend["wb"] = pp.tile(
                                [128, QC], F32, tag="pp", name=f"wb{qi}"
                            )
                        if gi < 16:
                            # PE duty filler near the chunk boundary.
                            wb = pend["wb"]
                            nc.tensor.matmul(
                                wb, lhsT=scratchR[:, 0:128],
                                rhs=scratchR, start=True, stop=True,
                                skip_group_check=True,
                            )
                    ps = pl.tile([128, GRP, QC], F32, tag="lg")
                    eT = eTp.tile([128, GRP, QC], BF16)
                    g0 = GRP * gi
                    for j in range(GRP):
                        kc = g0 + j
                        s = kc % 4
                        sl = slice(32 * s, 32 * (s + 1))
                        nc.tensor.matmul(
                            ps[:, j, :],
                            lhsT=g_sb[sl, ts(kc, KC)],
                            rhs=f_sb[sl, ts(qi, QC)],
                            start=True, stop=True,
                            tile_position=(32 * s, 0),
                        )
                    if gi in EXP_ACT:
                        nc.scalar.activation(out=eT, in_=ps, func=EXP)
                    else:
                        nc.vector.tensor_scalar(
                            out=eT.bitcast(I16), in0=ps,
                            scalar1=SCH_A, scalar2=SCH_B,
                            op0=MULT, op1=ADD,
                        )
                    x0q.append((g0, eT))
                    if len(x0q) > 2:
                        emit_x0(*x0q.pop(0))
                for g0, eT in x0q:
                    emit_x0(g0, eT)
                pend["x0t"] = x0t
                emit_norm_copies(qi)

            # tail: normalize + project the last chunk, with PE filler
            # bridging the norm-chain latency
            wbt = pp.tile([128, QC], F32, tag="pp", name="wbt")
            emit_norm_recip(NQC - 1, fast=True)
            for _ in range(22):
                nc.tensor.matmul(
                    wbt, lhsT=scratchR[:, 0:128], rhs=scratchR,
                    start=True, stop=True, skip_group_check=True,
                )
            emit_norm_mul(NQC - 1)
            emit_v_out(NQC - 1)

    nc.compile()
    return nc


def prep_inputs(x, wf, bf, wg, bg, wh, bh, wv, bv, gamma):
    """Host-side sharding/weight prep; returns per-core input maps."""
    x = np.asarray(x, dtype=np.float32)
    wf = np.asarray(wf, dtype=np.float32)
    bf = np.asarray(bf, dtype=np.float32)
    wg = np.asarray(wg, dtype=np.float32)
    bg = np.asarray(bg, dtype=np.float32)
    wh = np.asarray(wh, dtype=np.float32)
    bh = np.asarray(bh, dtype=np.float32)
    wv = np.asarray(wv, dtype=np.float32)
    bv = np.asarray(bv, dtype=np.float32)
    g0 = float(np.asarray(gamma, dtype=np.float32).reshape(-1)[0])

    bf16 = ml_dtypes.bfloat16
    xf = np.ascontiguousarray(x.reshape(B, C, N))
    # f/g weights replicated 4x along M so f/g land replicated on the
    # four 32-partition strips (enables row-packed logits matmuls).
    wfT = np.ascontiguousarray(np.tile(wf.T, (1, 4))).astype(bf16)
    wgT = np.ascontiguousarray(np.tile(wg.T, (1, 4))).astype(bf16)
    whT = np.ascontiguousarray(wh.T).astype(bf16)
    # v weights NEGATED (the on-chip Newton reciprocal yields -1/den);
    # the bias row is applied separately in the residual STT.
    wv2 = np.ascontiguousarray(-g0 * wv.T).astype(bf16)
    bv2 = np.ascontiguousarray(
        (g0 * (bv + wv @ bh)).reshape(2, 128).T
    ).astype(np.float32)
    bf4 = np.ascontiguousarray(np.tile(bf, 4).reshape(128, 1))
    bg4 = np.ascontiguousarray(np.tile(bg, 4).reshape(128, 1))

    in_maps = []
    for core in range(NCORES):
        b, half = divmod(core, 2)
        xb = xf[b]
        if half:
            xb = np.ascontiguousarray(
                np.concatenate([xb[:, NQ:], xb[:, :NQ]], axis=1)
            )
        in_maps.append(
            {"xbf": xb.astype(bf16), "wfT": wfT, "wgT": wgT, "whT": whT,
             "wv2": wv2, "bv2": bv2, "bf": bf4, "bg": bg4}
        )
    return in_maps


def kernel(x, wf, bf, wg, bg, wh, bh, wv, bv, gamma):
    global _cached_nc, LAST_EXEC_NS
    if _cached_nc is None:
        _cached_nc = _build()
    nc = _cached_nc
    in_maps = prep_inputs(x, wf, bf, wg, bg, wh, bh, wv, bv, gamma)

    res = run_bass_kernel_spmd(
        nc, in_maps, list(range(NCORES)),
        trace=TRACE or bool(os.environ.get("BASS_KERNEL_TRACE")),
    )
    LAST_EXEC_NS = res.exec_time_ns

    out = np.empty((B, C, N), np.float32)
    for core in range(NCORES):
        b, half = divmod(core, 2)
        out[b][:, half * NQ : (half + 1) * NQ] = res.results[core]["out"]
    return out.reshape(B, C, W, H)
